# revision 9
# baseline (speedup 1.0000x reference)
"""Trainium2 Bass kernel for nn_LinearAttention (B=8, C=256, H=W=64, 4 heads x 128).

Strategy
--------
Data-parallel over batch: each of the 8 NeuronCores processes one batch
element end-to-end (no collectives).

Per-core math (x is [C=256, n=4096]; all 1x1-conv weights pre-packed on host):

  Phase 1 (streamed over 32 n-tiles of 128):
    k^T tile = x_tile^T @ w_k^T          [128, 512]  (PSUM, 2 C-block accum)
    ek       = exp(k^T)                  (no max-subtraction; |k| <~ 5)
    A_h     += ek_h^T @ [x^T_tile | 1]   [128, 257]  accumulated over all
               tiles in PSUM; col 256 collects the softmax row-sums.

    This replaces the v-projection + context matmuls of the classic
    linear-attention pipeline: since ctx_h = (ek_h^T x^T) w_vh^T, the
    w_v factor is folded into the host-computed U_h = w_out_h @ w_vh.

  Phase 2 (tiny collapse, then one streamed matmul):
    Ahat_h = A_h / rowsum                 (per-partition scale on ACT)
    G_h    = Ahat_h^T-blocks @ w_qh       [256, 256]   (8 matmuls)
    W^T    = sum_h G_h^T-blocks @ U_h^T   [256, 256]   (16 matmuls)
    out    = W @ x + b                    [256, 4096]  streamed per
             512-col chunk; both output row-blocks ship in one DMA.

All inputs are packed host-side into ONE [128, 20512] bf16 tensor laid
out in consumption order, so the whole input side is 6 large contiguous
DMAs (fewer queues + semaphores shortens the fixed NEFF teardown sweep).
Streaming matmuls use bf16 operands with fp32 PSUM accumulation. Output
is bf16 (upcast on host). Narrow (128-col) warm-up matmuls at kernel
start flip the HAM clock gate during the initial DMA wait.
"""

import numpy as np

HEADS = 4
DH = 128
C = 256
HID = 512
N = 4096
NT = N // 128  # 32 n-tiles
TCW = 257      # xt tile width: 256 channels + ones column
NCORES = 8

# ---- packed input column map (bf16 [128, INP_COLS]) ----
# segment order == DMA/consumption order
_XC_BASE = {}
_XT_BASE = {}


def _build_colmap():
    off = 0
    segs = []

    def seg(name, width):
        nonlocal off
        segs.append((name, off, width))
        base = off
        off += width
        return base

    wk = seg("wk", 2 * HID)
    _XC_BASE[0] = seg("xc0", 1024)
    for t in range(4):
        _XT_BASE[t] = seg(f"xt{t}", TCW)
    _XC_BASE[1] = seg("xc1", 1024)
    for t in range(4, 8):
        _XT_BASE[t] = seg(f"xt{t}", TCW)
    wq = seg("wq", HEADS * C)
    _XC_BASE[2] = seg("xc2", 1024)
    _XC_BASE[3] = seg("xc3", 1024)
    for t in range(8, 16):
        _XT_BASE[t] = seg(f"xt{t}", TCW)
    u = seg("u", 2 * HEADS * C)
    for c in range(4, 8):
        _XC_BASE[c] = seg(f"xc{c}", 1024)
    for t in range(16, 32):
        _XT_BASE[t] = seg(f"xt{t}", TCW)
    return wk, wq, u, off


_WK_BASE, _WQ_BASE, _U_BASE, INP_COLS = _build_colmap()
# input DMA split points (columns); first two small (gate the first matmul),
# the rest ~0.5-1.5 MiB contiguous transfers
_DMA_SPLITS = [
    0,
    _XC_BASE[0],
    _XT_BASE[0],
    _XT_BASE[4],
    _XC_BASE[2],
    _U_BASE,
    _XC_BASE[6],
    INP_COLS,
]

_BUILD_CACHE = {}


def _build_program():
    """Build + compile the SPMD Bass program (same NEFF for all 8 cores)."""
    from contextlib import ExitStack

    import concourse.bass as bass
    import concourse.tile as tile
    from concourse import bacc, mybir

    f32 = mybir.dt.float32
    bf16 = mybir.dt.bfloat16
    AFT = mybir.ActivationFunctionType

    nc = bacc.Bacc(
        "TRN2", target_bir_lowering=False, debug=False, num_devices=NCORES
    )

    inp_d = nc.dram_tensor("inp", [128, INP_COLS], bf16, kind="ExternalInput").ap()
    bb_d = nc.dram_tensor("bb", [128, 2], f32, kind="ExternalInput").ap()
    out_d = nc.dram_tensor("out", [C, N], bf16, kind="ExternalOutput").ap()

    with tile.TileContext(nc) as tc, ExitStack() as stack:
        const = stack.enter_context(tc.tile_pool(name="const", bufs=1))

        inp = const.tile([128, INP_COLS], bf16)
        bb_sb = const.tile([128, 2], f32)
        # zero tile for PE warm-up matmuls (no DMA dependency)
        zt = const.tile([128, 2 * 128], bf16)
        nc.gpsimd.memset(zt[:], 0.0)

        for s0, s1 in zip(_DMA_SPLITS, _DMA_SPLITS[1:]):
            nc.sync.dma_start(inp[:, s0:s1], inp_d[:, s0:s1])
        nc.sync.dma_start(bb_sb[:], bb_d[:])

        def xs(k, i):  # lhsT: x rows k-block, spatial tile i -> [128, 128]
            base = _XC_BASE[i // 4] + k * 512 + (i % 4) * 128
            return inp[:, base : base + 128]

        def xchunk(k, c):  # rhs: x rows k-block, 512-col chunk c
            base = _XC_BASE[c] + k * 512
            return inp[:, base : base + 512]

        def xt(t):  # rhs: [x^T tile t | ones] -> [128, 257]
            return inp[:, _XT_BASE[t] : _XT_BASE[t] + TCW]

        def wk(k):
            return inp[:, _WK_BASE + k * HID : _WK_BASE + (k + 1) * HID]

        def wqh(h):
            return inp[:, _WQ_BASE + h * C : _WQ_BASE + (h + 1) * C]

        def uj(j):
            return inp[:, _U_BASE + j * C : _U_BASE + (j + 1) * C]

        rsum = const.tile([128, HEADS], f32)
        ahat_sb = const.tile([128, HEADS * C], bf16)
        g_sb = const.tile([128, 2 * HEADS * C], bf16)
        w_sb = const.tile([128, 2 * C], bf16)

        # ---- Phase 1: k^T projection + exp + A accumulation ----
        with tc.tile_pool(name="ap", bufs=1, space="PSUM") as app, \
             tc.tile_pool(name="pkp", bufs=3, space="PSUM") as pkp, \
             tc.tile_pool(name="warmp", bufs=1, space="PSUM") as warmp, \
             tc.tile_pool(name="ekp", bufs=5) as ekp:
            a_ps = [app.tile([128, TCW], f32, name=f"a{h}") for h in range(HEADS)]

            # Narrow warm-up matmuls: ~3us of back-to-back PE activity flips
            # the HAM clock gate to K=8/8 while the first input DMA is still
            # in flight.
            warm = warmp.tile([128, 128], f32, name="warm")
            for _ in range(20):
                nc.tensor.matmul(warm[:], zt[:, 0:128], zt[:, 128:256])

            def emit_A(ek, i):
                for h in range(HEADS):
                    nc.tensor.matmul(
                        a_ps[h][:],
                        ek[:, h * 128 : (h + 1) * 128],
                        xt(i),
                        start=(i == 0),
                        stop=(i == NT - 1),
                        skip_group_check=True,
                    )

            pending = []
            for i in range(NT):
                pk = pkp.tile([128, HID], f32, name="pk")
                for k in range(2):
                    nc.tensor.matmul(
                        pk[:], xs(k, i), wk(k), start=(k == 0), stop=(k == 1)
                    )
                ek = ekp.tile([128, HID], bf16, name="ek")
                nc.scalar.activation(ek[:], pk[:], AFT.Exp)
                # software-pipeline the A matmuls a few tiles behind so the
                # tensor engine never stalls on the exp of the same tile
                pending.append((ek, i))
                if len(pending) > 3:
                    emit_A(*pending.pop(0))
            for p in pending:
                emit_A(*p)

            # ---- normalize A while the accumulator banks are open ----
            for h in range(HEADS):
                nc.vector.reciprocal(rsum[:, h : h + 1], a_ps[h][:, 256:257])
            for h in range(HEADS):
                # alternate ACT/DVE so the 4 scales run 2-wide
                if h % 2 == 0:
                    nc.scalar.mul(
                        ahat_sb[:, h * C : (h + 1) * C],
                        a_ps[h][:, 0:C],
                        rsum[:, h : h + 1],
                    )
                else:
                    nc.vector.tensor_scalar_mul(
                        ahat_sb[:, h * C : (h + 1) * C],
                        a_ps[h][:, 0:C],
                        rsum[:, h : h + 1],
                    )

        # ---- Phase 2: G/W collapse, final matmul ----
        with tc.tile_pool(name="p2p", bufs=2, space="PSUM") as p2p, \
             tc.tile_pool(name="fop", bufs=6) as fop:
            # G_h[c2-block] = Ahat_h[:, c2-block]^T-contract w_qh -> [128, 256]
            for h in range(HEADS):
                for blk in range(2):
                    j = h * 2 + blk
                    gp = p2p.tile([128, C], f32, name="gp")
                    nc.tensor.matmul(
                        gp[:],
                        ahat_sb[:, h * C + blk * 128 : h * C + blk * 128 + 128],
                        wqh(h),
                    )
                    if j % 2 == 0:
                        nc.scalar.copy(g_sb[:, j * C : (j + 1) * C], gp[:])
                    else:
                        nc.vector.tensor_copy(g_sb[:, j * C : (j + 1) * C], gp[:])
            # W^T[cq-block m] = sum_{h,blk} G[h,blk][:, m-block]^T-contract U_h^T
            for m in range(2):
                wp = p2p.tile([128, C], f32, name="wp")
                for j in range(2 * HEADS):
                    nc.tensor.matmul(
                        wp[:],
                        g_sb[:, j * C + m * 128 : j * C + m * 128 + 128],
                        uj(j),
                        start=(j == 0),
                        stop=(j == 2 * HEADS - 1),
                    )
                if m == 0:
                    nc.scalar.copy(w_sb[:, m * C : (m + 1) * C], wp[:])
                else:
                    nc.vector.tensor_copy(w_sb[:, m * C : (m + 1) * C], wp[:])

            # out = W @ x + b, streamed per 512-col chunk; both output
            # row-blocks ride in one [p, 2, 512] DMA per chunk so stores
            # start early and the drain tail stays short.
            out3 = out_d.rearrange("(m p) n -> p m n", m=2)
            for c in range(8):
                fo = fop.tile([128, 1024], bf16, name="fo")
                for mo in range(2):
                    fp_ = p2p.tile([128, 512], f32, name="fp", bufs=3)
                    for k in range(2):
                        nc.tensor.matmul(
                            fp_[:],
                            w_sb[:, k * C + mo * 128 : k * C + mo * 128 + 128],
                            xchunk(k, c),
                            start=(k == 0),
                            stop=(k == 1),
                        )
                    half = fo[:, mo * 512 : (mo + 1) * 512]
                    if mo == 0:
                        nc.scalar.activation(
                            half, fp_[:], AFT.Identity, bias=bb_sb[:, 0:1]
                        )
                    else:
                        nc.vector.tensor_scalar_add(half, fp_[:], bb_sb[:, 1:2])
                nc.sync.dma_start(
                    out3[:, :, c * 512 : (c + 1) * 512],
                    fo.rearrange("p (m n) -> p m n", m=2),
                )

    nc.compile()
    return nc


def _get_program():
    if "nc" not in _BUILD_CACHE:
        _BUILD_CACHE["nc"] = _build_program()
    return _BUILD_CACHE["nc"]


def _pack_weights(w_qkv, w_out, b_out):
    """Weight portion of the packed input (same for every core)."""
    w_q = np.ascontiguousarray(w_qkv[0:HID]).astype(np.float32)  # [512, 256]
    w_k = np.ascontiguousarray(w_qkv[HID : 2 * HID]).astype(np.float32)
    w_v = np.ascontiguousarray(w_qkv[2 * HID : 3 * HID]).astype(np.float32)
    w_out = np.asarray(w_out, np.float32)

    segs = {}
    # wk: w_k.T [256, 512] -> [128, 2 C-blocks, 512]
    segs[_WK_BASE] = w_k.T.reshape(2, 128, HID).transpose(1, 0, 2).reshape(
        128, 2 * HID
    )
    # wq: rows of head h -> [128 d, h, 256]
    segs[_WQ_BASE] = (
        w_q.reshape(HEADS, 128, C).transpose(1, 0, 2).reshape(128, HEADS * C)
    )
    # U_h = w_out_h @ w_vh [256 o, 256 c_in]; store U_h^T row-blocks
    u = np.empty((128, 2 * HEADS * C), np.float32)
    for h in range(HEADS):
        UhT = (w_out[:, h * DH : (h + 1) * DH] @ w_v[h * DH : (h + 1) * DH]).T
        for blk in range(2):
            u[:, (2 * h + blk) * C : (2 * h + blk + 1) * C] = UhT[
                blk * 128 : (blk + 1) * 128
            ]
    segs[_U_BASE] = u

    bb = np.ascontiguousarray(
        np.asarray(b_out, np.float32).reshape(2, 128).T
    ).astype(np.float32)
    return segs, bb


def _pack_inp(xb_f32, wsegs):
    """Per-batch packed input [128, INP_COLS] bf16."""
    import ml_dtypes

    inp = np.zeros((128, INP_COLS), np.float32)
    for base, seg in wsegs.items():
        inp[:, base : base + seg.shape[1]] = seg
    xr = xb_f32.reshape(2, 128, 8, 512)  # [k, p, c, 512]
    for c in range(8):
        b = _XC_BASE[c]
        inp[:, b : b + 512] = xr[0, :, c]
        inp[:, b + 512 : b + 1024] = xr[1, :, c]
    xt3 = xb_f32.reshape(C, NT, 128)  # [c, t, p]
    for t in range(NT):
        b = _XT_BASE[t]
        inp[:, b : b + C] = xt3[:, t, :].T
        inp[:, b + C] = 1.0
    return np.ascontiguousarray(inp).astype(ml_dtypes.bfloat16)


def _make_inmaps(x, w_qkv, w_out, b_out):
    wsegs, bb = _pack_weights(
        np.asarray(w_qkv, np.float32),
        np.asarray(w_out, np.float32),
        np.asarray(b_out, np.float32),
    )
    x = np.asarray(x, dtype=np.float32)
    return [
        {"inp": _pack_inp(x[b].reshape(C, N), wsegs), "bb": bb}
        for b in range(x.shape[0])
    ]


def _ensure_ntff_hook():
    """Make trace-mode grading (BASS_TRACE=1) work even when the container's
    ``antenv`` stub lacks ``axon_hooks``: install the registry module and, if
    the axon PJRT library is present, register the ctypes NTFF profile hook."""
    import os
    import sys
    import types

    try:
        import antenv.axon_hooks  # noqa: F401
    except ImportError:
        try:
            import antenv
        except ImportError:
            return
        mod = types.ModuleType("antenv.axon_hooks")
        mod._hook = None
        mod.set_axon_ntff_profile_hook = lambda h: setattr(mod, "_hook", h)
        mod.get_axon_ntff_profile_hook = lambda: getattr(mod, "_hook", None)
        sys.modules["antenv.axon_hooks"] = mod
        antenv.axon_hooks = mod
    try:
        from antenv.axon_hooks import (
            get_axon_ntff_profile_hook,
            set_axon_ntff_profile_hook,
        )

        so = "/opt/axon/libaxon_pjrt.so"
        if get_axon_ntff_profile_hook() is None and os.path.exists(so):
            from trn_agent_boot.trn_boot import _ntff_profile_via_ctypes

            hook = _ntff_profile_via_ctypes(so)
            if hook is not None:
                set_axon_ntff_profile_hook(hook)
    except Exception:
        pass


def kernel(x, w_qkv, w_out, b_out):
    from concourse.bass_utils import run_bass_kernel_spmd

    _ensure_ntff_hook()

    x = np.asarray(x, dtype=np.float32)
    B = x.shape[0]
    assert B == NCORES and x.shape[1:] == (C, 64, 64)

    nc = _get_program()
    in_maps = _make_inmaps(x, w_qkv, w_out, b_out)
    res = run_bass_kernel_spmd(nc, in_maps, core_ids=list(range(NCORES)))
    out = np.stack(
        [np.asarray(res.results[b]["out"], dtype=np.float32) for b in range(B)], axis=0
    )
    return out.reshape(B, C, 64, 64).astype(np.float32)


# revision 11
# speedup vs baseline: 1.0063x; 1.0063x over previous
"""Trainium2 Bass kernel for nn_LinearAttention (B=8, C=256, H=W=64, 4 heads x 128).

Strategy
--------
Data-parallel over batch: each of the 8 NeuronCores processes one batch
element end-to-end (no collectives).

Per-core math (x is [C=256, n=4096]; all 1x1-conv weights pre-packed on host):

  Phase 1 (streamed over 32 n-tiles of 128):
    k^T tile = x_tile^T @ w_k^T          [128, 512]  (PSUM, 2 C-block accum)
    ek       = exp(k^T)                  (no max-subtraction; |k| <~ 5)
    A_h     += ek_h^T @ [x^T_tile | 1]   [128, 257]  accumulated over all
               tiles in PSUM; col 256 collects the softmax row-sums.

    This replaces the v-projection + context matmuls of the classic
    linear-attention pipeline: since ctx_h = (ek_h^T x^T) w_vh^T, the
    w_v factor is folded into the host-computed U_h = w_out_h @ w_vh.

  Phase 2 (tiny collapse, then one streamed matmul):
    Ahat_h = A_h / rowsum                 (per-partition scale on ACT)
    G_h    = Ahat_h^T-blocks @ w_qh       [256, 256]   (8 matmuls)
    W^T    = sum_h G_h^T-blocks @ U_h^T   [256, 256]   (16 matmuls)
    out    = W @ x + b                    [256, 4096]  streamed per
             512-col chunk; both output row-blocks ship in one DMA.

All inputs are packed host-side into ONE [128, 20512] bf16 tensor laid
out in consumption order, so the whole input side is 6 large contiguous
DMAs (fewer queues + semaphores shortens the fixed NEFF teardown sweep).
Streaming matmuls use bf16 operands with fp32 PSUM accumulation. Output
is bf16 (upcast on host). Narrow (128-col) warm-up matmuls at kernel
start flip the HAM clock gate during the initial DMA wait.
"""

import numpy as np

HEADS = 4
DH = 128
C = 256
HID = 512
N = 4096
NT = N // 128  # 32 n-tiles
TCW = 257      # xt tile width: 256 channels + ones column
NCORES = 8

# ---- packed input column map (bf16 [128, INP_COLS]) ----
# segment order == DMA/consumption order
_XC_BASE = {}
_XT_BASE = {}


def _build_colmap():
    off = 0
    segs = []

    def seg(name, width):
        nonlocal off
        segs.append((name, off, width))
        base = off
        off += width
        return base

    wk = seg("wk", 2 * HID)
    _XC_BASE[0] = seg("xc0", 1024)
    for t in range(4):
        _XT_BASE[t] = seg(f"xt{t}", TCW)
    _XC_BASE[1] = seg("xc1", 1024)
    for t in range(4, 8):
        _XT_BASE[t] = seg(f"xt{t}", TCW)
    wq = seg("wq", HEADS * C)
    _XC_BASE[2] = seg("xc2", 1024)
    _XC_BASE[3] = seg("xc3", 1024)
    for t in range(8, 16):
        _XT_BASE[t] = seg(f"xt{t}", TCW)
    u = seg("u", 2 * HEADS * C)
    for c in range(4, 8):
        _XC_BASE[c] = seg(f"xc{c}", 1024)
    for t in range(16, 32):
        _XT_BASE[t] = seg(f"xt{t}", TCW)
    return wk, wq, u, off


_WK_BASE, _WQ_BASE, _U_BASE, INP_COLS = _build_colmap()
# input DMA split points (columns); first two small (gate the first matmul),
# the rest ~0.5-1.5 MiB contiguous transfers
_DMA_SPLITS = [
    0,
    _XC_BASE[0],
    _XT_BASE[0],
    _XT_BASE[4],
    _XC_BASE[2],
    _U_BASE,
    _XC_BASE[6],
    INP_COLS,
]

_BUILD_CACHE = {}


def _build_program():
    """Build + compile the SPMD Bass program (same NEFF for all 8 cores)."""
    from contextlib import ExitStack

    import concourse.bass as bass
    import concourse.tile as tile
    from concourse import bacc, mybir

    f32 = mybir.dt.float32
    bf16 = mybir.dt.bfloat16
    AFT = mybir.ActivationFunctionType

    nc = bacc.Bacc(
        "TRN2", target_bir_lowering=False, debug=False, num_devices=NCORES
    )

    inp_d = nc.dram_tensor("inp", [128, INP_COLS], bf16, kind="ExternalInput").ap()
    bb_d = nc.dram_tensor("bb", [128, 2], f32, kind="ExternalInput").ap()
    out_d = nc.dram_tensor("out", [C, N], bf16, kind="ExternalOutput").ap()

    with tile.TileContext(nc) as tc, ExitStack() as stack:
        const = stack.enter_context(tc.tile_pool(name="const", bufs=1))

        inp = const.tile([128, INP_COLS], bf16)
        bb_sb = const.tile([128, 2], f32)
        # zero tile for PE warm-up matmuls (no DMA dependency)
        zt = const.tile([128, 2 * 128], bf16)
        nc.gpsimd.memset(zt[:], 0.0)

        # first two segments gate the first matmuls: issue them in parallel
        # on the two HWDGE rings (Sync + Scalar) to halve the startup
        # completion-receipt latency; everything else streams on Sync.
        for i, (s0, s1) in enumerate(zip(_DMA_SPLITS, _DMA_SPLITS[1:])):
            eng = nc.scalar if i in (1, 2) else nc.sync
            eng.dma_start(inp[:, s0:s1], inp_d[:, s0:s1])
        nc.sync.dma_start(bb_sb[:], bb_d[:])

        def xs(k, i):  # lhsT: x rows k-block, spatial tile i -> [128, 128]
            base = _XC_BASE[i // 4] + k * 512 + (i % 4) * 128
            return inp[:, base : base + 128]

        def xchunk(k, c):  # rhs: x rows k-block, 512-col chunk c
            base = _XC_BASE[c] + k * 512
            return inp[:, base : base + 512]

        def xt(t):  # rhs: [x^T tile t | ones] -> [128, 257]
            return inp[:, _XT_BASE[t] : _XT_BASE[t] + TCW]

        def wk(k):
            return inp[:, _WK_BASE + k * HID : _WK_BASE + (k + 1) * HID]

        def wqh(h):
            return inp[:, _WQ_BASE + h * C : _WQ_BASE + (h + 1) * C]

        def uj(j):
            return inp[:, _U_BASE + j * C : _U_BASE + (j + 1) * C]

        rsum = const.tile([128, HEADS], f32)
        ahat_sb = const.tile([128, HEADS * C], bf16)
        g_sb = const.tile([128, 2 * HEADS * C], bf16)
        w_sb = const.tile([128, 2 * C], bf16)

        # ---- Phase 1: k^T projection + exp + A accumulation ----
        with tc.tile_pool(name="ap", bufs=1, space="PSUM") as app, \
             tc.tile_pool(name="pkp", bufs=3, space="PSUM") as pkp, \
             tc.tile_pool(name="warmp", bufs=1, space="PSUM") as warmp, \
             tc.tile_pool(name="ekp", bufs=5) as ekp:
            a_ps = [app.tile([128, TCW], f32, name=f"a{h}") for h in range(HEADS)]

            # Narrow warm-up matmuls: ~3us of back-to-back PE activity flips
            # the HAM clock gate to K=8/8 while the first input DMA is still
            # in flight.
            warm = warmp.tile([128, 128], f32, name="warm")
            for _ in range(32):
                nc.tensor.matmul(warm[:], zt[:, 0:128], zt[:, 128:256])

            def emit_A(ek, i):
                for h in range(HEADS):
                    nc.tensor.matmul(
                        a_ps[h][:],
                        ek[:, h * 128 : (h + 1) * 128],
                        xt(i),
                        start=(i == 0),
                        stop=(i == NT - 1),
                        skip_group_check=True,
                    )

            pending = []
            for i in range(NT):
                pk = pkp.tile([128, HID], f32, name="pk")
                for k in range(2):
                    nc.tensor.matmul(
                        pk[:], xs(k, i), wk(k), start=(k == 0), stop=(k == 1)
                    )
                ek = ekp.tile([128, HID], bf16, name="ek")
                nc.scalar.activation(ek[:], pk[:], AFT.Exp)
                # software-pipeline the A matmuls a few tiles behind so the
                # tensor engine never stalls on the exp of the same tile
                pending.append((ek, i))
                if len(pending) > 3:
                    emit_A(*pending.pop(0))
            for p in pending:
                emit_A(*p)

            # ---- normalize A while the accumulator banks are open ----
            for h in range(HEADS):
                nc.vector.reciprocal(rsum[:, h : h + 1], a_ps[h][:, 256:257])
            for h in range(HEADS):
                # alternate ACT/DVE so the 4 scales run 2-wide
                if h % 2 == 0:
                    nc.scalar.mul(
                        ahat_sb[:, h * C : (h + 1) * C],
                        a_ps[h][:, 0:C],
                        rsum[:, h : h + 1],
                    )
                else:
                    nc.vector.tensor_scalar_mul(
                        ahat_sb[:, h * C : (h + 1) * C],
                        a_ps[h][:, 0:C],
                        rsum[:, h : h + 1],
                    )

        # ---- Phase 2: G/W collapse, final matmul ----
        with tc.tile_pool(name="p2p", bufs=2, space="PSUM") as p2p, \
             tc.tile_pool(name="fop", bufs=6) as fop:
            # G_h[c2-block] = Ahat_h[:, c2-block]^T-contract w_qh -> [128, 256]
            for h in range(HEADS):
                for blk in range(2):
                    j = h * 2 + blk
                    gp = p2p.tile([128, C], f32, name="gp")
                    nc.tensor.matmul(
                        gp[:],
                        ahat_sb[:, h * C + blk * 128 : h * C + blk * 128 + 128],
                        wqh(h),
                    )
                    if j % 2 == 0:
                        nc.scalar.copy(g_sb[:, j * C : (j + 1) * C], gp[:])
                    else:
                        nc.vector.tensor_copy(g_sb[:, j * C : (j + 1) * C], gp[:])
            # W^T[cq-block m] = sum_{h,blk} G[h,blk][:, m-block]^T-contract U_h^T
            for m in range(2):
                wp = p2p.tile([128, C], f32, name="wp")
                for j in range(2 * HEADS):
                    nc.tensor.matmul(
                        wp[:],
                        g_sb[:, j * C + m * 128 : j * C + m * 128 + 128],
                        uj(j),
                        start=(j == 0),
                        stop=(j == 2 * HEADS - 1),
                    )
                if m == 0:
                    nc.scalar.copy(w_sb[:, m * C : (m + 1) * C], wp[:])
                else:
                    nc.vector.tensor_copy(w_sb[:, m * C : (m + 1) * C], wp[:])

            # out = W @ x + b, streamed per 512-col chunk; both output
            # row-blocks ride in one [p, 2, 512] DMA per chunk so stores
            # start early and the drain tail stays short.
            out3 = out_d.rearrange("(m p) n -> p m n", m=2)
            for c in range(8):
                fo = fop.tile([128, 1024], bf16, name="fo")
                for mo in range(2):
                    fp_ = p2p.tile([128, 512], f32, name="fp", bufs=3)
                    for k in range(2):
                        nc.tensor.matmul(
                            fp_[:],
                            w_sb[:, k * C + mo * 128 : k * C + mo * 128 + 128],
                            xchunk(k, c),
                            start=(k == 0),
                            stop=(k == 1),
                        )
                    half = fo[:, mo * 512 : (mo + 1) * 512]
                    if mo == 0:
                        nc.scalar.activation(
                            half, fp_[:], AFT.Identity, bias=bb_sb[:, 0:1]
                        )
                    else:
                        nc.vector.tensor_scalar_add(half, fp_[:], bb_sb[:, 1:2])
                nc.sync.dma_start(
                    out3[:, :, c * 512 : (c + 1) * 512],
                    fo.rearrange("p (m n) -> p m n", m=2),
                )

    nc.compile()
    return nc


def _get_program():
    if "nc" not in _BUILD_CACHE:
        _BUILD_CACHE["nc"] = _build_program()
    return _BUILD_CACHE["nc"]


def _pack_weights(w_qkv, w_out, b_out):
    """Weight portion of the packed input (same for every core)."""
    w_q = np.ascontiguousarray(w_qkv[0:HID]).astype(np.float32)  # [512, 256]
    w_k = np.ascontiguousarray(w_qkv[HID : 2 * HID]).astype(np.float32)
    w_v = np.ascontiguousarray(w_qkv[2 * HID : 3 * HID]).astype(np.float32)
    w_out = np.asarray(w_out, np.float32)

    segs = {}
    # wk: w_k.T [256, 512] -> [128, 2 C-blocks, 512]
    segs[_WK_BASE] = w_k.T.reshape(2, 128, HID).transpose(1, 0, 2).reshape(
        128, 2 * HID
    )
    # wq: rows of head h -> [128 d, h, 256]
    segs[_WQ_BASE] = (
        w_q.reshape(HEADS, 128, C).transpose(1, 0, 2).reshape(128, HEADS * C)
    )
    # U_h = w_out_h @ w_vh [256 o, 256 c_in]; store U_h^T row-blocks
    u = np.empty((128, 2 * HEADS * C), np.float32)
    for h in range(HEADS):
        UhT = (w_out[:, h * DH : (h + 1) * DH] @ w_v[h * DH : (h + 1) * DH]).T
        for blk in range(2):
            u[:, (2 * h + blk) * C : (2 * h + blk + 1) * C] = UhT[
                blk * 128 : (blk + 1) * 128
            ]
    segs[_U_BASE] = u

    bb = np.ascontiguousarray(
        np.asarray(b_out, np.float32).reshape(2, 128).T
    ).astype(np.float32)
    return segs, bb


def _pack_inp(xb_f32, wsegs):
    """Per-batch packed input [128, INP_COLS] bf16."""
    import ml_dtypes

    inp = np.zeros((128, INP_COLS), np.float32)
    for base, seg in wsegs.items():
        inp[:, base : base + seg.shape[1]] = seg
    xr = xb_f32.reshape(2, 128, 8, 512)  # [k, p, c, 512]
    for c in range(8):
        b = _XC_BASE[c]
        inp[:, b : b + 512] = xr[0, :, c]
        inp[:, b + 512 : b + 1024] = xr[1, :, c]
    xt3 = xb_f32.reshape(C, NT, 128)  # [c, t, p]
    for t in range(NT):
        b = _XT_BASE[t]
        inp[:, b : b + C] = xt3[:, t, :].T
        inp[:, b + C] = 1.0
    return np.ascontiguousarray(inp).astype(ml_dtypes.bfloat16)


def _make_inmaps(x, w_qkv, w_out, b_out):
    wsegs, bb = _pack_weights(
        np.asarray(w_qkv, np.float32),
        np.asarray(w_out, np.float32),
        np.asarray(b_out, np.float32),
    )
    x = np.asarray(x, dtype=np.float32)
    return [
        {"inp": _pack_inp(x[b].reshape(C, N), wsegs), "bb": bb}
        for b in range(x.shape[0])
    ]


def _ensure_ntff_hook():
    """Make trace-mode grading (BASS_TRACE=1) work even when the container's
    ``antenv`` stub lacks ``axon_hooks``: install the registry module and, if
    the axon PJRT library is present, register the ctypes NTFF profile hook."""
    import os
    import sys
    import types

    try:
        import antenv.axon_hooks  # noqa: F401
    except ImportError:
        try:
            import antenv
        except ImportError:
            return
        mod = types.ModuleType("antenv.axon_hooks")
        mod._hook = None
        mod.set_axon_ntff_profile_hook = lambda h: setattr(mod, "_hook", h)
        mod.get_axon_ntff_profile_hook = lambda: getattr(mod, "_hook", None)
        sys.modules["antenv.axon_hooks"] = mod
        antenv.axon_hooks = mod
    try:
        from antenv.axon_hooks import (
            get_axon_ntff_profile_hook,
            set_axon_ntff_profile_hook,
        )

        so = "/opt/axon/libaxon_pjrt.so"
        if get_axon_ntff_profile_hook() is None and os.path.exists(so):
            from trn_agent_boot.trn_boot import _ntff_profile_via_ctypes

            hook = _ntff_profile_via_ctypes(so)
            if hook is not None:
                set_axon_ntff_profile_hook(hook)
    except Exception:
        pass


def kernel(x, w_qkv, w_out, b_out):
    from concourse.bass_utils import run_bass_kernel_spmd

    _ensure_ntff_hook()

    x = np.asarray(x, dtype=np.float32)
    B = x.shape[0]
    assert B == NCORES and x.shape[1:] == (C, 64, 64)

    nc = _get_program()
    in_maps = _make_inmaps(x, w_qkv, w_out, b_out)
    res = run_bass_kernel_spmd(nc, in_maps, core_ids=list(range(NCORES)))
    out = np.stack(
        [np.asarray(res.results[b]["out"], dtype=np.float32) for b in range(B)], axis=0
    )
    return out.reshape(B, C, 64, 64).astype(np.float32)
